# revision 1
# baseline (speedup 1.0000x reference)
"""Dense MoE (all-experts, gate-weighted sum) on 8 Trainium2 NeuronCores.

Sharding: pure data-parallel over the token axis N (8192 -> 1024 rows/core);
every core holds all 8 experts, so no collectives are needed.

Math folded per core (N_loc=1024, D=1024, E=8, O=1024, H=256):
    h      = relu(x @ W_g1.T + b_g1)                 # gating MLP, bf16 matmuls
    gates  = softmax(h @ W_g2.T + b_g2)              # fp32 softmax
    out    = sum_e gates[:,e] * (x @ W_e[e].T) + gates @ b_e

The expert GEMMs accumulate over D in PSUM (fp32); the gate weighting + sum
over experts is ACT mul (per-partition gate scale) + DVE add per tile.
The bias term rides a tiny K=8 matmul (gates.T as stationary operand),
overlapped with expert 1's GEMM stream.

All matmul operands are bf16 (host-cast); accumulation fp32. Measured on
hardware: ~265 us per core (PE-bound; bf16 N=512 matmul roofline is ~221 us),
rel err vs fp32 reference ~2.3e-3 absmax-relative.
"""

import numpy as np
import ml_dtypes

import concourse.bass as bass
import concourse.mybir as mybir
import concourse.tile as tile
from concourse.bass_utils import run_bass_kernel_spmd

N, D, E, O, H = 8192, 1024, 8, 1024, 256
NCORES = 8
NLOC = N // NCORES          # 1024 rows per core
P = 128                     # partitions
NT = NLOC // P              # 8 n-tiles
DK = D // P                 # 8 contraction tiles
FO = 512                    # matmul moving free dim (one PSUM bank of fp32)
OH = O // FO                # 2 output halves
H2 = H // P                 # 2 h-tiles
BF16 = mybir.dt.bfloat16
F32 = mybir.dt.float32
BF = ml_dtypes.bfloat16


def legalize_single_wait(nc, max_waits=1):
    """This walrus build rejects instructions carrying more than one sync
    wait. Split each multi-wait instruction: excess waits move onto fresh
    same-engine NoOps inserted immediately before it (identical semantics:
    the engine stalls at the same program point on every semaphore)."""
    for f in nc.m.functions:
        for blk in f.blocks:
            insts = list(blk.instructions)
            if all(
                (i.sync_info is None or len(i.sync_info.on_wait) <= max_waits)
                for i in insts
            ):
                continue
            new = []
            for inst in insts:
                si = inst.sync_info
                if si is not None and len(si.on_wait) > max_waits:
                    waits = list(si.on_wait)
                    for k, w in enumerate(waits[:-max_waits]):
                        nop = mybir.InstNoOp(name=f"{inst.name}-w{k}")
                        nop.engine = inst.engine
                        nop.sync_info = mybir.SyncInfo(on_wait=[w], on_update=[])
                        new.append(nop)
                    si.on_wait = waits[-max_waits:]
                new.append(inst)
            blk.instructions = new
    return nc


def build_moe():
    nc = bass.Bass(target_bir_lowering=False)
    xT = nc.dram_tensor("xT", [D, NLOC], BF16, kind="ExternalInput")
    wt = nc.dram_tensor("wt", [E, D, O], BF16, kind="ExternalInput")
    wg1t = nc.dram_tensor("wg1t", [D, H], BF16, kind="ExternalInput")
    wg2t = nc.dram_tensor("wg2t", [H, E], BF16, kind="ExternalInput")
    bg1 = nc.dram_tensor("bg1", [H], F32, kind="ExternalInput")
    bg2 = nc.dram_tensor("bg2", [E], BF16, kind="ExternalInput")
    be = nc.dram_tensor("be", [E, O], BF16, kind="ExternalInput")
    ident = nc.dram_tensor("ident", [P, P], F32, kind="ExternalInput")
    out = nc.dram_tensor("out", [NLOC, O], F32, kind="ExternalOutput")

    with tile.TileContext(nc) as tc:
        with (
            tc.tile_pool(name="const", bufs=1) as constp,
            tc.tile_pool(name="wpool", bufs=4) as wpool,
            tc.tile_pool(name="work", bufs=4) as workp,
            tc.tile_pool(name="pro_ps", bufs=2, space="PSUM") as prop,
            tc.tile_pool(name="bias_ps", bufs=1, space="PSUM") as biasp,
            tc.tile_pool(name="mm_ps", bufs=5, space="PSUM") as mmp,
        ):
            # ---- PE warm-up: dummy matmuls on memset tiles (no DMA deps)
            # keep the PE busy while the first transfers land, so the HAM
            # clock-gate reaches 2.4 GHz before real work arrives ----
            warm_a = constp.tile([P, P], BF16, tag="warm_a")
            nc.vector.memset(warm_a, 0.0)
            warm_b = constp.tile([P, FO], BF16, tag="warm_b")
            nc.vector.memset(warm_b, 0.0)
            for i in range(24):
                wpsum = mmp.tile([P, FO], F32, tag="mm", name=f"warm{i}")
                nc.tensor.matmul(wpsum, warm_a, warm_b, start=True, stop=True)

            # ---- resident inputs (gating-critical transfers first, per-dk
            # interleaved so the first gating matmuls start ASAP) ----
            wg1t_sb = [
                constp.tile([P, H], BF16, tag=f"wg1t{dk}", name=f"wg1t{dk}")
                for dk in range(DK)
            ]
            xT_sb = [
                constp.tile([P, NLOC], BF16, tag=f"xTd{dk}", name=f"xTd{dk}")
                for dk in range(DK)
            ]
            wt0_r = wt[0].rearrange("(dk p) o -> p dk o", p=P)
            w0_half = []
            for oh in range(OH):
                wh = wpool.tile([P, DK, FO], BF16, tag=f"wh{oh}", name=f"wh{oh}")
                w0_half.append(wh)
            for dk in range(DK):
                nc.sync.dma_start(
                    out=wg1t_sb[dk], in_=wg1t[dk * P : (dk + 1) * P, :]
                )
                nc.sync.dma_start(
                    out=xT_sb[dk], in_=xT[dk * P : (dk + 1) * P, :]
                )
                nc.sync.dma_start(
                    out=w0_half[0][:, dk, :], in_=wt0_r[:, dk, 0:FO]
                )
            nc.sync.dma_start(out=w0_half[1], in_=wt0_r[:, :, FO : 2 * FO])
            wg2t_sb = constp.tile([P, H2, E], BF16, tag="wg2t")
            nc.gpsimd.dma_start(
                out=wg2t_sb, in_=wg2t.rearrange("(h2 p) e -> p h2 e", p=P)
            )
            bg1_sb = constp.tile([P, H2], F32, tag="bg1")
            nc.gpsimd.dma_start(out=bg1_sb, in_=bg1.rearrange("(h2 p) -> p h2", p=P))
            bg2_sb = constp.tile([1, E], BF16, tag="bg2")
            nc.gpsimd.dma_start(out=bg2_sb, in_=bg2[:])
            be_sb = constp.tile([E, O], BF16, tag="be")
            nc.gpsimd.dma_start(out=be_sb, in_=be[:, :])
            ones_sb = constp.tile([1, P], BF16, tag="ones")
            nc.vector.memset(ones_sb, 1.0)
            ident_sb = constp.tile([P, P], F32, tag="ident")
            nc.gpsimd.dma_start(out=ident_sb, in_=ident[:, :])

            # ---- gating: hT[h, n] = relu(W_g1 @ x.T + b_g1) ----
            hT_sb = [
                constp.tile([P, NLOC], BF16, tag=f"hT{h2}", name=f"hT{h2}") for h2 in range(H2)
            ]
            psum_g = {
                (h2, nh): mmp.tile([P, FO], F32, tag="mm", name=f"psum_g{h2}_{nh}")
                for h2 in range(H2)
                for nh in range(NLOC // FO)
            }
            for dk in range(DK):
                for h2 in range(H2):
                    for nh in range(NLOC // FO):
                        nc.tensor.matmul(
                            psum_g[(h2, nh)],
                            wg1t_sb[dk][:, h2 * P : (h2 + 1) * P],
                            xT_sb[dk][:, nh * FO : (nh + 1) * FO],
                            start=(dk == 0),
                            stop=(dk == DK - 1),
                        )
            for h2 in range(H2):
                for nh in range(NLOC // FO):
                    nc.scalar.activation(
                        out=hT_sb[h2][:, nh * FO : (nh + 1) * FO],
                        in_=psum_g[(h2, nh)],
                        func=mybir.ActivationFunctionType.Relu,
                        bias=bg1_sb[:, h2 : h2 + 1],
                    )

            # ---- gating: logits -> softmax -> gates, gates.T ----
            gates_sb = []
            gatesT_sb = []
            for nt in range(NT):
                psum_l = prop.tile([P, E], F32, tag="pro")
                for h2 in range(H2):
                    nc.tensor.matmul(
                        psum_l,
                        hT_sb[h2][:, nt * P : (nt + 1) * P],
                        wg2t_sb[:, h2, :],
                        start=(h2 == 0),
                        stop=False,
                    )
                nc.tensor.matmul(psum_l, ones_sb, bg2_sb, start=False, stop=True)

                negmax = workp.tile([P, 1], F32, tag="negmax")
                nc.vector.reduce_max(
                    negmax, psum_l, axis=mybir.AxisListType.X, negate=True
                )
                gates = constp.tile([P, E], F32, tag=f"gates{nt}", name=f"gates{nt}")
                sumexp = workp.tile([P, 1], F32, tag="sumexp")
                nc.scalar.activation(
                    out=gates,
                    in_=psum_l,
                    func=mybir.ActivationFunctionType.Exp,
                    bias=negmax,
                    accum_out=sumexp,
                )
                rsum = workp.tile([P, 1], F32, tag="rsum")
                nc.vector.reciprocal(rsum, sumexp)
                nc.vector.tensor_scalar_mul(gates, gates, rsum)
                gates_sb.append(gates)

            acc_sb = [
                [
                    constp.tile(
                        [P, FO], F32, tag=f"acc{nt}_{oh}", name=f"acc{nt}_{oh}"
                    )
                    for oh in range(OH)
                ]
                for nt in range(NT)
            ]

            # ---- main loop: stream experts, accumulate gate-weighted GEMM ----
            for e in range(E):
                if e == 0:
                    w_half = w0_half
                else:
                    wt_r = wt[e].rearrange("(dk p) o -> p dk o", p=P)
                    w_half = []
                    for oh in range(OH):
                        wh = wpool.tile(
                            [P, DK, FO], BF16, tag=f"wh{oh}", name=f"wh{oh}"
                        )
                        nc.sync.dma_start(
                            out=wh, in_=wt_r[:, :, oh * FO : (oh + 1) * FO]
                        )
                        w_half.append(wh)
                for oh in range(OH):
                    for nt in range(NT):
                        psum = mmp.tile([P, FO], F32, tag="mm")
                        for dk in range(DK):
                            nc.tensor.matmul(
                                psum,
                                xT_sb[dk][:, nt * P : (nt + 1) * P],
                                w_half[oh][:, dk, :],
                                start=(dk == 0),
                                stop=(dk == DK - 1),
                            )
                        acc = acc_sb[nt][oh]
                        if e == 0:
                            nc.scalar.mul(acc, psum, gates_sb[nt][:, e : e + 1])
                        else:
                            tmp = workp.tile([P, FO], F32, tag="tmp", name="tmp")
                            nc.scalar.mul(tmp, psum, gates_sb[nt][:, e : e + 1])
                            nc.vector.tensor_add(acc, acc, tmp)
                        if e == E - 1:
                            nc.scalar.dma_start(
                                out=out[
                                    nt * P : (nt + 1) * P, oh * FO : (oh + 1) * FO
                                ],
                                in_=acc,
                            )
                if e == 0:
                    # gates.T + bias matmuls — emitted here so the PE work
                    # hides inside experts 0-1's dense matmul stream and the
                    # kernel tail stays short
                    for nt in range(NT):
                        psum_t = prop.tile([E, P], F32, tag="pro", name="psum_t")
                        nc.tensor.transpose(psum_t, gates_sb[nt], ident_sb)
                        gatesT = constp.tile(
                            [E, P], BF16, tag=f"gatesT{nt}", name=f"gatesT{nt}"
                        )
                        nc.scalar.copy(out=gatesT, in_=psum_t)
                        gatesT_sb.append(gatesT)
                if e == 1:
                    for nt in range(NT):
                        for boh in range(OH):
                            psum_b = biasp.tile(
                                [P, FO], F32, tag="bias", name="psum_b"
                            )
                            nc.tensor.matmul(
                                psum_b,
                                gatesT_sb[nt],
                                be_sb[:, boh * FO : (boh + 1) * FO],
                                start=True,
                                stop=True,
                            )
                            nc.vector.tensor_add(
                                acc_sb[nt][boh], acc_sb[nt][boh], psum_b
                            )

    legalize_single_wait(nc)
    return nc


_NC_CACHE = {}


def _get_nc():
    if "nc" not in _NC_CACHE:
        _NC_CACHE["nc"] = build_moe()
    return _NC_CACHE["nc"]


def make_in_maps(x, W_e, b_e, W_g1, b_g1, W_g2, b_g2):
    x = np.asarray(x, dtype=np.float32)
    wt = np.ascontiguousarray(
        np.asarray(W_e, dtype=np.float32).transpose(0, 2, 1)
    ).astype(BF)
    wg1t = np.ascontiguousarray(np.asarray(W_g1, dtype=np.float32).T).astype(BF)
    wg2t = np.ascontiguousarray(np.asarray(W_g2, dtype=np.float32).T).astype(BF)
    bg1 = np.asarray(b_g1, dtype=np.float32)
    bg2 = np.asarray(b_g2, dtype=np.float32).astype(BF)
    be = np.asarray(b_e, dtype=np.float32).astype(BF)
    xb = x.astype(BF)
    ident_np = np.eye(P, dtype=np.float32)
    in_maps = []
    for c in range(NCORES):
        xT_c = np.ascontiguousarray(xb[c * NLOC : (c + 1) * NLOC, :].T)
        in_maps.append(
            {
                "xT": xT_c,
                "wt": wt,
                "wg1t": wg1t,
                "wg2t": wg2t,
                "bg1": bg1,
                "bg2": bg2,
                "be": be,
                "ident": ident_np,
            }
        )
    return in_maps


def kernel(x, W_e, b_e, W_g1, b_g1, W_g2, b_g2, **run_kwargs):
    nc = _get_nc()
    in_maps = make_in_maps(x, W_e, b_e, W_g1, b_g1, W_g2, b_g2)
    res = run_bass_kernel_spmd(nc, in_maps, core_ids=list(range(NCORES)), **run_kwargs)
    out = np.concatenate([res.results[c]["out"] for c in range(NCORES)], axis=0)
    if run_kwargs:
        kernel.last_results = res
    return out


if __name__ == "__main__":
    rng = np.random.default_rng(0)
    s = 1.0 / np.sqrt(D)
    sh = 1.0 / np.sqrt(H)
    inputs = {
        "x": rng.standard_normal((N, D), dtype=np.float32),
        "W_e": rng.uniform(-s, s, (E, O, D)).astype(np.float32),
        "b_e": rng.uniform(-s, s, (E, O)).astype(np.float32),
        "W_g1": rng.uniform(-s, s, (H, D)).astype(np.float32),
        "b_g1": rng.uniform(-s, s, (H,)).astype(np.float32),
        "W_g2": rng.uniform(-sh, sh, (E, H)).astype(np.float32),
        "b_g2": rng.uniform(-sh, sh, (E,)).astype(np.float32),
    }
    out = kernel(**inputs)
    print("out", out.shape, out.dtype, float(np.abs(out).max()))



# revision 2
# speedup vs baseline: 1.0006x; 1.0006x over previous
"""Dense MoE (all-experts, gate-weighted sum) on 8 Trainium2 NeuronCores.

Sharding: pure data-parallel over the token axis N (8192 -> 1024 rows/core);
every core holds all 8 experts, so no collectives are needed.

Math folded per core (N_loc=1024, D=1024, E=8, O=1024, H=256):
    h      = relu(x @ W_g1.T + b_g1)                 # gating MLP, bf16 matmuls
    gates  = softmax(h @ W_g2.T + b_g2)              # fp32 softmax
    out    = sum_e gates[:,e] * (x @ W_e[e].T) + gates @ b_e

v2 schedule (vs the ~265us baseline):
  - no big dummy-warmup block: 8 tiny N=128 matmuls prime the HAM clock
    gate while the first DMAs land, then the gating GEMM itself runs and
    finishes the warmup;
  - DMA queues split: sync queue carries the gating-critical xT/W_g1
    stream then experts 1-7; the scalar queue carries expert 0's weights
    in parallel; gpsimd carries the small constants;
  - relu is emitted per psum-group so logits can start ~3us earlier, and
    expert-0 matmul groups are interleaved with the logits/softmax phase
    so the PE never idles there;
  - the 16 gate.T @ b_e bias matmuls share the main PSUM pool and are
    interleaved into expert 1's stream (the dedicated 1-buf pool used to
    serialize the PE for ~5us during expert 2);
  - expert 7's epilogue runs in half-tiles and streams the output DMA on
    the sync queue to shorten the kernel tail.

All matmul operands are bf16 (host-cast); accumulation fp32.
"""

import numpy as np
import ml_dtypes

import concourse.bass as bass
import concourse.mybir as mybir
import concourse.tile as tile
from concourse.bass_utils import run_bass_kernel_spmd

N, D, E, O, H = 8192, 1024, 8, 1024, 256
NCORES = 8
NLOC = N // NCORES          # 1024 rows per core
P = 128                     # partitions
NT = NLOC // P              # 8 n-tiles
DK = D // P                 # 8 contraction tiles
FO = 512                    # matmul moving free dim (one PSUM bank of fp32)
OH = O // FO                # 2 output halves
H2 = H // P                 # 2 h-tiles
BF16 = mybir.dt.bfloat16
F32 = mybir.dt.float32
BF = ml_dtypes.bfloat16


def legalize_single_wait(nc, max_waits=1):
    """This walrus build rejects instructions carrying more than one sync
    wait. Split each multi-wait instruction: excess waits move onto fresh
    same-engine NoOps inserted immediately before it (identical semantics:
    the engine stalls at the same program point on every semaphore)."""
    for f in nc.m.functions:
        for blk in f.blocks:
            insts = list(blk.instructions)
            if all(
                (i.sync_info is None or len(i.sync_info.on_wait) <= max_waits)
                for i in insts
            ):
                continue
            new = []
            for inst in insts:
                si = inst.sync_info
                if si is not None and len(si.on_wait) > max_waits:
                    waits = list(si.on_wait)
                    for k, w in enumerate(waits[:-max_waits]):
                        nop = mybir.InstNoOp(name=f"{inst.name}-w{k}")
                        nop.engine = inst.engine
                        nop.sync_info = mybir.SyncInfo(on_wait=[w], on_update=[])
                        new.append(nop)
                    si.on_wait = waits[-max_waits:]
                new.append(inst)
            blk.instructions = new
    return nc


def build_moe():
    nc = bass.Bass(target_bir_lowering=False)
    xT = nc.dram_tensor("xT", [D, NLOC], BF16, kind="ExternalInput")
    wt = nc.dram_tensor("wt", [E, D, O], BF16, kind="ExternalInput")
    wg1t = nc.dram_tensor("wg1t", [D, H], BF16, kind="ExternalInput")
    wg2t = nc.dram_tensor("wg2t", [H, E], BF16, kind="ExternalInput")
    bg1 = nc.dram_tensor("bg1", [H], F32, kind="ExternalInput")
    bg2 = nc.dram_tensor("bg2", [E], BF16, kind="ExternalInput")
    be = nc.dram_tensor("be", [E, O], BF16, kind="ExternalInput")
    ident = nc.dram_tensor("ident", [P, P], F32, kind="ExternalInput")
    out = nc.dram_tensor("out", [NLOC, O], F32, kind="ExternalOutput")

    with tile.TileContext(nc) as tc:
        with (
            tc.tile_pool(name="const", bufs=1) as constp,
            tc.tile_pool(name="wpool", bufs=4) as wpool,
            tc.tile_pool(name="work", bufs=4) as workp,
            tc.tile_pool(name="pro_ps", bufs=3, space="PSUM") as prop,
            tc.tile_pool(name="mm_ps", bufs=5, space="PSUM") as mmp,
        ):
            # ---- tiny PE warm-up: N=128 matmuls on memset tiles keep the
            # HAM activity window busy while the first transfers land ----
            warm_a = constp.tile([P, P], BF16, tag="warm_a")
            nc.vector.memset(warm_a, 0.0)
            warm_b = constp.tile([P, P], BF16, tag="warm_b")
            nc.vector.memset(warm_b, 0.0)
            for i in range(8):
                wpsum = mmp.tile([P, FO], F32, tag="mm", name=f"warm{i}")
                nc.tensor.matmul(
                    wpsum[:, 0:P], warm_a, warm_b, start=True, stop=True
                )

            # ---- resident inputs. sync queue: gating-critical stream
            # (wg1t/xT per-dk interleaved), experts 1-7 follow later.
            # scalar queue: expert-0 weights in parallel. ----
            wg1t_sb = [
                constp.tile([P, H], BF16, tag=f"wg1t{dk}", name=f"wg1t{dk}")
                for dk in range(DK)
            ]
            xT_sb = [
                constp.tile([P, NLOC], BF16, tag=f"xTd{dk}", name=f"xTd{dk}")
                for dk in range(DK)
            ]
            for dk in range(DK):
                nc.sync.dma_start(
                    out=wg1t_sb[dk], in_=wg1t[dk * P : (dk + 1) * P, :]
                )
                nc.sync.dma_start(
                    out=xT_sb[dk], in_=xT[dk * P : (dk + 1) * P, :]
                )
            wt0_r = wt[0].rearrange("(dk p) o -> p dk o", p=P)
            w0_half = []
            for oh in range(OH):
                wh = wpool.tile([P, DK, FO], BF16, tag=f"wh{oh}", name=f"wh{oh}")
                nc.scalar.dma_start(
                    out=wh, in_=wt0_r[:, :, oh * FO : (oh + 1) * FO]
                )
                w0_half.append(wh)
            wg2t_sb = constp.tile([P, H2, E], BF16, tag="wg2t")
            nc.gpsimd.dma_start(
                out=wg2t_sb, in_=wg2t.rearrange("(h2 p) e -> p h2 e", p=P)
            )
            bg1_sb = constp.tile([P, H2], F32, tag="bg1")
            nc.gpsimd.dma_start(out=bg1_sb, in_=bg1.rearrange("(h2 p) -> p h2", p=P))
            bg2_sb = constp.tile([1, E], BF16, tag="bg2")
            nc.gpsimd.dma_start(out=bg2_sb, in_=bg2[:])
            be_sb = constp.tile([E, O], BF16, tag="be")
            nc.gpsimd.dma_start(out=be_sb, in_=be[:, :])
            ones_sb = constp.tile([1, P], BF16, tag="ones")
            nc.vector.memset(ones_sb, 1.0)
            ident_sb = constp.tile([P, P], F32, tag="ident")
            nc.gpsimd.dma_start(out=ident_sb, in_=ident[:, :])

            # ---- gating: hT[h, n] = relu(W_g1 @ x.T + b_g1) ----
            # dk 0..6 interleaved across the 4 psum groups (starts as soon
            # as each dk chunk lands); dk=7 per group with relu emitted
            # immediately so hT becomes available incrementally.
            hT_sb = [
                constp.tile([P, NLOC], BF16, tag=f"hT{h2}", name=f"hT{h2}")
                for h2 in range(H2)
            ]
            NH = NLOC // FO
            groups = [(0, 0), (1, 0), (0, 1), (1, 1)]  # (h2, nh): nh=0 first
            psum_g = {
                g: mmp.tile([P, FO], F32, tag="mm", name=f"psum_g{g[0]}_{g[1]}")
                for g in groups
            }
            for dk in range(DK - 1):
                for h2, nh in groups:
                    nc.tensor.matmul(
                        psum_g[(h2, nh)],
                        wg1t_sb[dk][:, h2 * P : (h2 + 1) * P],
                        xT_sb[dk][:, nh * FO : (nh + 1) * FO],
                        start=(dk == 0),
                        stop=False,
                    )
            for h2, nh in groups:
                nc.tensor.matmul(
                    psum_g[(h2, nh)],
                    wg1t_sb[DK - 1][:, h2 * P : (h2 + 1) * P],
                    xT_sb[DK - 1][:, nh * FO : (nh + 1) * FO],
                    start=False,
                    stop=True,
                )
                nc.scalar.activation(
                    out=hT_sb[h2][:, nh * FO : (nh + 1) * FO],
                    in_=psum_g[(h2, nh)],
                    func=mybir.ActivationFunctionType.Relu,
                    bias=bg1_sb[:, h2 : h2 + 1],
                )

            # ---- gating: logits -> softmax -> gates (per nt-tile) ----
            gates_sb = [None] * NT
            gatesT_sb = [None] * NT

            def emit_logits_softmax(nt):
                psum_l = prop.tile([P, E], F32, tag="pro")
                for h2 in range(H2):
                    nc.tensor.matmul(
                        psum_l,
                        hT_sb[h2][:, nt * P : (nt + 1) * P],
                        wg2t_sb[:, h2, :],
                        start=(h2 == 0),
                        stop=False,
                    )
                nc.tensor.matmul(psum_l, ones_sb, bg2_sb, start=False, stop=True)
                negmax = workp.tile([P, 1], F32, tag="negmax")
                nc.vector.reduce_max(
                    negmax, psum_l, axis=mybir.AxisListType.X, negate=True
                )
                gates = constp.tile([P, E], F32, tag=f"gates{nt}", name=f"gates{nt}")
                sumexp = workp.tile([P, 1], F32, tag="sumexp")
                nc.scalar.activation(
                    out=gates,
                    in_=psum_l,
                    func=mybir.ActivationFunctionType.Exp,
                    bias=negmax,
                    accum_out=sumexp,
                )
                rsum = workp.tile([P, 1], F32, tag="rsum")
                nc.vector.reciprocal(rsum, sumexp)
                nc.vector.tensor_scalar_mul(gates, gates, rsum)
                gates_sb[nt] = gates

            acc_sb = [
                [
                    constp.tile(
                        [P, FO], F32, tag=f"acc{nt}_{oh}", name=f"acc{nt}_{oh}"
                    )
                    for oh in range(OH)
                ]
                for nt in range(NT)
            ]

            # ---- expert-0 matmul groups (no epilogue yet) ----
            def emit_expert_group_mms(w_half, oh, nt):
                psum = mmp.tile([P, FO], F32, tag="mm")
                for dk in range(DK):
                    nc.tensor.matmul(
                        psum,
                        xT_sb[dk][:, nt * P : (nt + 1) * P],
                        w_half[oh][:, dk, :],
                        start=(dk == 0),
                        stop=(dk == DK - 1),
                    )
                return psum

            def emit_mul(psum, e, oh, nt):
                acc = acc_sb[nt][oh]
                if e == 0:
                    nc.scalar.mul(acc, psum, gates_sb[nt][:, e : e + 1])
                else:
                    tmp = workp.tile([P, FO], F32, tag="tmp", name="tmp")
                    nc.scalar.mul(tmp, psum, gates_sb[nt][:, e : e + 1])
                    nc.vector.tensor_add(acc, acc, tmp)

            # expert-0, interleaved with logits/softmax/transposes.
            # psum groups EG0..EG2 are emitted before their ACT muls so the
            # ACT FIFO stays: relu x4 -> exp x8 -> muls (no FIFO deadlock).
            e0_tiles = [(oh, nt) for oh in range(OH) for nt in range(NT)]
            pend = []  # groups whose mul is deferred
            pend.append((emit_expert_group_mms(w0_half, *e0_tiles[0]), e0_tiles[0]))
            pend.append((emit_expert_group_mms(w0_half, *e0_tiles[1]), e0_tiles[1]))
            for nt in range(4):
                emit_logits_softmax(nt)
            pend.append((emit_expert_group_mms(w0_half, *e0_tiles[2]), e0_tiles[2]))
            for nt in range(4, NT):
                emit_logits_softmax(nt)
            for psum, (oh, nt) in pend:
                emit_mul(psum, 0, oh, nt)
            for k in (3, 4):
                oh, nt = e0_tiles[k]
                psum = emit_expert_group_mms(w0_half, oh, nt)
                emit_mul(psum, 0, oh, nt)
            # gates.T via PE transpose (hides inside expert-0's stream)
            for nt in range(NT):
                psum_t = prop.tile([E, P], F32, tag="pro", name="psum_t")
                nc.tensor.transpose(psum_t, gates_sb[nt], ident_sb)
                gatesT = constp.tile(
                    [E, P], BF16, tag=f"gatesT{nt}", name=f"gatesT{nt}"
                )
                nc.scalar.copy(out=gatesT, in_=psum_t)
                gatesT_sb[nt] = gatesT
            for k in range(5, len(e0_tiles)):
                oh, nt = e0_tiles[k]
                psum = emit_expert_group_mms(w0_half, oh, nt)
                emit_mul(psum, 0, oh, nt)

            # ---- experts 1-7: stream weights on sync queue, accumulate
            # gate-weighted GEMM. bias (gates.T @ b_e) groups share the mm
            # psum pool and interleave into expert 1's stream. ----
            for e in range(1, E):
                wt_r = wt[e].rearrange("(dk p) o -> p dk o", p=P)
                w_half = []
                for oh in range(OH):
                    wh = wpool.tile(
                        [P, DK, FO], BF16, tag=f"wh{oh}", name=f"wh{oh}"
                    )
                    nc.sync.dma_start(
                        out=wh, in_=wt_r[:, :, oh * FO : (oh + 1) * FO]
                    )
                    w_half.append(wh)
                for oh in range(OH):
                    for nt in range(NT):
                        psum = emit_expert_group_mms(w_half, oh, nt)
                        if e == E - 1:
                            # final expert: half-tile epilogue + streamed
                            # output DMA to shorten the kernel tail
                            acc = acc_sb[nt][oh]
                            tmp = workp.tile([P, FO], F32, tag="tmp", name="tmp")
                            for hh in range(2):
                                sl = slice(hh * (FO // 2), (hh + 1) * (FO // 2))
                                nc.scalar.mul(
                                    tmp[:, sl], psum[:, sl],
                                    gates_sb[nt][:, e : e + 1],
                                )
                                nc.vector.tensor_add(
                                    acc[:, sl], acc[:, sl], tmp[:, sl]
                                )
                                nc.sync.dma_start(
                                    out=out[
                                        nt * P : (nt + 1) * P,
                                        oh * FO + hh * (FO // 2)
                                        : oh * FO + (hh + 1) * (FO // 2),
                                    ],
                                    in_=acc[:, sl],
                                )
                        else:
                            emit_mul(psum, e, oh, nt)
                        if e == 1:
                            # one bias group per expert-1 group
                            bi = oh * NT + nt
                            boh, bnt = bi // NT, bi % NT
                            psum_b = mmp.tile([P, FO], F32, tag="mm", name="psum_b")
                            nc.tensor.matmul(
                                psum_b,
                                gatesT_sb[bnt],
                                be_sb[:, boh * FO : (boh + 1) * FO],
                                start=True,
                                stop=True,
                            )
                            nc.vector.tensor_add(
                                acc_sb[bnt][boh], acc_sb[bnt][boh], psum_b
                            )

    legalize_single_wait(nc)
    return nc


_NC_CACHE = {}


def _get_nc():
    if "nc" not in _NC_CACHE:
        _NC_CACHE["nc"] = build_moe()
    return _NC_CACHE["nc"]


def make_in_maps(x, W_e, b_e, W_g1, b_g1, W_g2, b_g2):
    x = np.asarray(x, dtype=np.float32)
    wt = np.ascontiguousarray(
        np.asarray(W_e, dtype=np.float32).transpose(0, 2, 1)
    ).astype(BF)
    wg1t = np.ascontiguousarray(np.asarray(W_g1, dtype=np.float32).T).astype(BF)
    wg2t = np.ascontiguousarray(np.asarray(W_g2, dtype=np.float32).T).astype(BF)
    bg1 = np.asarray(b_g1, dtype=np.float32)
    bg2 = np.asarray(b_g2, dtype=np.float32).astype(BF)
    be = np.asarray(b_e, dtype=np.float32).astype(BF)
    xb = x.astype(BF)
    ident_np = np.eye(P, dtype=np.float32)
    in_maps = []
    for c in range(NCORES):
        xT_c = np.ascontiguousarray(xb[c * NLOC : (c + 1) * NLOC, :].T)
        in_maps.append(
            {
                "xT": xT_c,
                "wt": wt,
                "wg1t": wg1t,
                "wg2t": wg2t,
                "bg1": bg1,
                "bg2": bg2,
                "be": be,
                "ident": ident_np,
            }
        )
    return in_maps


def kernel(x, W_e, b_e, W_g1, b_g1, W_g2, b_g2, **run_kwargs):
    nc = _get_nc()
    in_maps = make_in_maps(x, W_e, b_e, W_g1, b_g1, W_g2, b_g2)
    res = run_bass_kernel_spmd(nc, in_maps, core_ids=list(range(NCORES)), **run_kwargs)
    out = np.concatenate([res.results[c]["out"] for c in range(NCORES)], axis=0)
    if run_kwargs:
        kernel.last_results = res
    return out


if __name__ == "__main__":
    rng = np.random.default_rng(0)
    s = 1.0 / np.sqrt(D)
    sh = 1.0 / np.sqrt(H)
    inputs = {
        "x": rng.standard_normal((N, D), dtype=np.float32),
        "W_e": rng.uniform(-s, s, (E, O, D)).astype(np.float32),
        "b_e": rng.uniform(-s, s, (E, O)).astype(np.float32),
        "W_g1": rng.uniform(-s, s, (H, D)).astype(np.float32),
        "b_g1": rng.uniform(-sh, sh, (H,)).astype(np.float32),
        "W_g2": rng.uniform(-sh, sh, (E, H)).astype(np.float32),
        "b_g2": rng.uniform(-sh, sh, (E,)).astype(np.float32),
    }
    out = kernel(**inputs)
    print("out", out.shape, out.dtype, float(np.abs(out).max()))


# revision 5
# speedup vs baseline: 1.0149x; 1.0144x over previous
"""Dense MoE (all-experts, gate-weighted sum) on 8 Trainium2 NeuronCores.

Sharding: pure data-parallel over the token axis N (8192 -> 1024 rows/core);
every core holds all 8 experts, so no collectives are needed.

Math folded per core (N_loc=1024, D=1024, E=8, O=1024, H=256):
    h      = relu(x @ W_g1.T + b_g1)                 # gating MLP, bf16 matmuls
    gates  = softmax(h @ W_g2.T + b_g2)              # fp32 softmax
    out    = sum_e gates[:,e] * (x @ W_e[e].T) + gates @ b_e

v2 schedule (vs the ~265us baseline):
  - no big dummy-warmup block: 8 tiny N=128 matmuls prime the HAM clock
    gate while the first DMAs land, then the gating GEMM itself runs and
    finishes the warmup;
  - DMA queues split: sync queue carries the gating-critical xT/W_g1
    stream then experts 1-7; the scalar queue carries expert 0's weights
    in parallel; gpsimd carries the small constants;
  - relu is emitted per psum-group so logits can start ~3us earlier, and
    expert-0 matmul groups are interleaved with the logits/softmax phase
    so the PE never idles there;
  - the 16 gate.T @ b_e bias matmuls share the main PSUM pool and are
    interleaved into expert 1's stream (the dedicated 1-buf pool used to
    serialize the PE for ~5us during expert 2);
  - expert 7's epilogue runs in half-tiles and streams the output DMA on
    the sync queue to shorten the kernel tail.

All matmul operands are bf16 (host-cast); accumulation fp32.
"""

import numpy as np
import ml_dtypes

import concourse.bass as bass
import concourse.mybir as mybir
import concourse.tile as tile
from concourse.bass_utils import run_bass_kernel_spmd

N, D, E, O, H = 8192, 1024, 8, 1024, 256
NCORES = 8
NLOC = N // NCORES          # 1024 rows per core
P = 128                     # partitions
NT = NLOC // P              # 8 n-tiles
DK = D // P                 # 8 contraction tiles
FO = 512                    # matmul moving free dim (one PSUM bank of fp32)
OH = O // FO                # 2 output halves
H2 = H // P                 # 2 h-tiles
BF16 = mybir.dt.bfloat16
F32 = mybir.dt.float32
BF = ml_dtypes.bfloat16


def legalize_single_wait(nc, max_waits=1):
    """This walrus build rejects instructions carrying more than one sync
    wait. Split each multi-wait instruction: excess waits move onto fresh
    same-engine NoOps inserted immediately before it (identical semantics:
    the engine stalls at the same program point on every semaphore)."""
    for f in nc.m.functions:
        for blk in f.blocks:
            insts = list(blk.instructions)
            if all(
                (i.sync_info is None or len(i.sync_info.on_wait) <= max_waits)
                for i in insts
            ):
                continue
            new = []
            for inst in insts:
                si = inst.sync_info
                if si is not None and len(si.on_wait) > max_waits:
                    waits = list(si.on_wait)
                    for k, w in enumerate(waits[:-max_waits]):
                        nop = mybir.InstNoOp(name=f"{inst.name}-w{k}")
                        nop.engine = inst.engine
                        nop.sync_info = mybir.SyncInfo(on_wait=[w], on_update=[])
                        new.append(nop)
                    si.on_wait = waits[-max_waits:]
                new.append(inst)
            blk.instructions = new
    return nc


def build_moe():
    nc = bass.Bass(target_bir_lowering=False)
    xT = nc.dram_tensor("xT", [D, NLOC], BF16, kind="ExternalInput")
    wt = nc.dram_tensor("wt", [E, D, O], BF16, kind="ExternalInput")
    wg1t = nc.dram_tensor("wg1t", [D, H], BF16, kind="ExternalInput")
    wg2t = nc.dram_tensor("wg2t", [H, E], BF16, kind="ExternalInput")
    bg1 = nc.dram_tensor("bg1", [H], F32, kind="ExternalInput")
    bg2 = nc.dram_tensor("bg2", [E], BF16, kind="ExternalInput")
    be = nc.dram_tensor("be", [E, O], BF16, kind="ExternalInput")
    ident = nc.dram_tensor("ident", [P, P], F32, kind="ExternalInput")
    out = nc.dram_tensor("out", [NLOC, O], F32, kind="ExternalOutput")

    with tile.TileContext(nc) as tc:
        with (
            tc.tile_pool(name="const", bufs=1) as constp,
            tc.tile_pool(name="wpool", bufs=4) as wpool,
            tc.tile_pool(name="work", bufs=4) as workp,
            tc.tile_pool(name="pro_ps", bufs=3, space="PSUM") as prop,
            tc.tile_pool(name="mm_ps", bufs=5, space="PSUM") as mmp,
        ):
            # ---- tiny PE warm-up: N=128 matmuls on memset tiles keep the
            # HAM activity window busy while the first transfers land ----
            warm_a = constp.tile([P, P], BF16, tag="warm_a")
            nc.vector.memset(warm_a, 0.0)
            warm_b = constp.tile([P, P], BF16, tag="warm_b")
            nc.vector.memset(warm_b, 0.0)
            for i in range(14):
                wpsum = mmp.tile([P, FO], F32, tag="mm", name=f"warm{i}")
                nc.tensor.matmul(
                    wpsum[:, 0:P], warm_a, warm_b, start=True, stop=True
                )

            # ---- resident inputs. ONE hardware queue (sync) carries every
            # input in priority order: the DMA fabric tops out ~350 GB/s
            # per core, so a second queue only steals bandwidth from the
            # gating-critical stream. Order: (wg1t,xT) per dk -> expert-0
            # weights (half 0 per-dk so expert-0 matmuls can start before
            # the half finishes) -> experts 1-7 (pool-gated). ----
            wg1t_sb = [
                constp.tile([P, H], BF16, tag=f"wg1t{dk}", name=f"wg1t{dk}")
                for dk in range(DK)
            ]
            xT_sb = [
                constp.tile([P, NLOC], BF16, tag=f"xTd{dk}", name=f"xTd{dk}")
                for dk in range(DK)
            ]
            for dk in range(DK):
                nc.sync.dma_start(
                    out=wg1t_sb[dk], in_=wg1t[dk * P : (dk + 1) * P, :]
                )
                nc.sync.dma_start(
                    out=xT_sb[dk], in_=xT[dk * P : (dk + 1) * P, :]
                )
            wt0_r = wt[0].rearrange("(dk p) o -> p dk o", p=P)
            w0_half = [
                wpool.tile([P, DK, FO], BF16, tag=f"wh{oh}", name=f"wh{oh}")
                for oh in range(OH)
            ]
            for dk in range(DK):
                nc.sync.dma_start(
                    out=w0_half[0][:, dk, :], in_=wt0_r[:, dk, 0:FO]
                )
            nc.sync.dma_start(out=w0_half[1], in_=wt0_r[:, :, FO : 2 * FO])
            wg2t_sb = constp.tile([P, H2, E], BF16, tag="wg2t")
            nc.gpsimd.dma_start(
                out=wg2t_sb, in_=wg2t.rearrange("(h2 p) e -> p h2 e", p=P)
            )
            bg1_sb = constp.tile([P, H2], F32, tag="bg1")
            nc.gpsimd.dma_start(out=bg1_sb, in_=bg1.rearrange("(h2 p) -> p h2", p=P))
            bg2_sb = constp.tile([1, E], BF16, tag="bg2")
            nc.gpsimd.dma_start(out=bg2_sb, in_=bg2[:])
            be_sb = constp.tile([E, O], BF16, tag="be")
            nc.gpsimd.dma_start(out=be_sb, in_=be[:, :])
            ones_sb = constp.tile([1, P], BF16, tag="ones")
            nc.vector.memset(ones_sb, 1.0)
            ident_sb = constp.tile([P, P], F32, tag="ident")
            nc.gpsimd.dma_start(out=ident_sb, in_=ident[:, :])

            # ---- gating: hT[h, n] = relu(W_g1 @ x.T + b_g1) ----
            # dk 0..6 interleaved across the 4 psum groups (starts as soon
            # as each dk chunk lands); dk=7 per group with relu emitted
            # immediately so hT becomes available incrementally.
            hT_sb = [
                constp.tile([P, NLOC], BF16, tag=f"hT{h2}", name=f"hT{h2}")
                for h2 in range(H2)
            ]
            NH = NLOC // FO
            groups = [(0, 0), (1, 0), (0, 1), (1, 1)]  # (h2, nh): nh=0 first
            psum_g = {
                g: mmp.tile([P, FO], F32, tag="mm", name=f"psum_g{g[0]}_{g[1]}")
                for g in groups
            }
            for dk in range(DK - 1):
                for h2, nh in groups:
                    nc.tensor.matmul(
                        psum_g[(h2, nh)],
                        wg1t_sb[dk][:, h2 * P : (h2 + 1) * P],
                        xT_sb[dk][:, nh * FO : (nh + 1) * FO],
                        start=(dk == 0),
                        stop=False,
                    )
            for h2, nh in groups:
                nc.tensor.matmul(
                    psum_g[(h2, nh)],
                    wg1t_sb[DK - 1][:, h2 * P : (h2 + 1) * P],
                    xT_sb[DK - 1][:, nh * FO : (nh + 1) * FO],
                    start=False,
                    stop=True,
                )
                nc.scalar.activation(
                    out=hT_sb[h2][:, nh * FO : (nh + 1) * FO],
                    in_=psum_g[(h2, nh)],
                    func=mybir.ActivationFunctionType.Relu,
                    bias=bg1_sb[:, h2 : h2 + 1],
                )

            # ---- gating: logits -> softmax -> gates (per nt-tile) ----
            gates_sb = [None] * NT
            gatesT_sb = [None] * NT

            def emit_logits_softmax(nt):
                psum_l = prop.tile([P, E], F32, tag="pro")
                for h2 in range(H2):
                    nc.tensor.matmul(
                        psum_l,
                        hT_sb[h2][:, nt * P : (nt + 1) * P],
                        wg2t_sb[:, h2, :],
                        start=(h2 == 0),
                        stop=False,
                    )
                nc.tensor.matmul(psum_l, ones_sb, bg2_sb, start=False, stop=True)
                negmax = workp.tile([P, 1], F32, tag="negmax")
                nc.vector.reduce_max(
                    negmax, psum_l, axis=mybir.AxisListType.X, negate=True
                )
                gates = constp.tile([P, E], F32, tag=f"gates{nt}", name=f"gates{nt}")
                sumexp = workp.tile([P, 1], F32, tag="sumexp")
                nc.scalar.activation(
                    out=gates,
                    in_=psum_l,
                    func=mybir.ActivationFunctionType.Exp,
                    bias=negmax,
                    accum_out=sumexp,
                )
                rsum = workp.tile([P, 1], F32, tag="rsum")
                nc.vector.reciprocal(rsum, sumexp)
                nc.vector.tensor_scalar_mul(gates, gates, rsum)
                gates_sb[nt] = gates

            acc_sb = [
                [
                    constp.tile(
                        [P, FO], F32, tag=f"acc{nt}_{oh}", name=f"acc{nt}_{oh}"
                    )
                    for oh in range(OH)
                ]
                for nt in range(NT)
            ]

            # ---- expert matmul groups ----
            def emit_expert_group_mms(w_half, oh, nt):
                psum = mmp.tile([P, FO], F32, tag="mm")
                for dk in range(DK):
                    nc.tensor.matmul(
                        psum,
                        xT_sb[dk][:, nt * P : (nt + 1) * P],
                        w_half[oh][:, dk, :],
                        start=(dk == 0),
                        stop=(dk == DK - 1),
                    )
                return psum

            def emit_mul(psum, e, oh, nt):
                acc = acc_sb[nt][oh]
                if e == 0:
                    nc.scalar.mul(acc, psum, gates_sb[nt][:, e : e + 1])
                else:
                    tmp = workp.tile([P, FO], F32, tag="tmp", name="tmp")
                    nc.scalar.mul(tmp, psum, gates_sb[nt][:, e : e + 1])
                    nc.vector.tensor_add(acc, acc, tmp)

            # expert-0 runs in 4-group batches with dk OUTER so the PE can
            # consume expert-0 weight chunks while they are still landing,
            # interleaved with logits/softmax. ACT-FIFO order stays:
            # relu x4 -> exp nt0-3 -> exp nt4-7 -> gate-muls (no deadlock).
            def emit_batch_dks(psums, oh, nts, dks):
                for dk in dks:
                    for i, nt in enumerate(nts):
                        nc.tensor.matmul(
                            psums[i],
                            xT_sb[dk][:, nt * P : (nt + 1) * P],
                            w0_half[oh][:, dk, :],
                            start=(dk == 0),
                            stop=(dk == DK - 1),
                        )

            b1 = [mmp.tile([P, FO], F32, tag="mm", name=f"b1_{i}") for i in range(4)]
            emit_batch_dks(b1, 0, range(4), range(0, 2))
            for nt in range(4):
                emit_logits_softmax(nt)
            emit_batch_dks(b1, 0, range(4), range(2, 5))
            for nt in range(4, NT):
                emit_logits_softmax(nt)
            emit_batch_dks(b1, 0, range(4), range(5, DK))
            for i, nt in enumerate(range(4)):
                emit_mul(b1[i], 0, 0, nt)
            b2 = [mmp.tile([P, FO], F32, tag="mm", name=f"b2_{i}") for i in range(4)]
            emit_batch_dks(b2, 0, range(4, NT), range(DK))
            for i, nt in enumerate(range(4, NT)):
                emit_mul(b2[i], 0, 0, nt)
            # gates.T via PE transpose (hides inside expert-0's stream)
            for nt in range(NT):
                psum_t = prop.tile([E, P], F32, tag="pro", name="psum_t")
                nc.tensor.transpose(psum_t, gates_sb[nt], ident_sb)
                gatesT = constp.tile(
                    [E, P], BF16, tag=f"gatesT{nt}", name=f"gatesT{nt}"
                )
                nc.scalar.copy(out=gatesT, in_=psum_t)
                gatesT_sb[nt] = gatesT
            for nt in range(NT):
                psum = emit_expert_group_mms(w0_half, 1, nt)
                emit_mul(psum, 0, 1, nt)

            # ---- experts 1-7: stream weights on sync queue, accumulate
            # gate-weighted GEMM. bias (gates.T @ b_e) groups share the mm
            # psum pool and interleave into expert 1's stream. ----
            for e in range(1, E):
                wt_r = wt[e].rearrange("(dk p) o -> p dk o", p=P)
                w_half = []
                for oh in range(OH):
                    wh = wpool.tile(
                        [P, DK, FO], BF16, tag=f"wh{oh}", name=f"wh{oh}"
                    )
                    nc.sync.dma_start(
                        out=wh, in_=wt_r[:, :, oh * FO : (oh + 1) * FO]
                    )
                    w_half.append(wh)
                for oh in range(OH):
                    for nt in range(NT):
                        psum = emit_expert_group_mms(w_half, oh, nt)
                        if e == E - 1:
                            # final expert: half-tile epilogue + streamed
                            # output DMA to shorten the kernel tail
                            acc = acc_sb[nt][oh]
                            tmp = workp.tile([P, FO], F32, tag="tmp", name="tmp")
                            for hh in range(2):
                                sl = slice(hh * (FO // 2), (hh + 1) * (FO // 2))
                                nc.scalar.mul(
                                    tmp[:, sl], psum[:, sl],
                                    gates_sb[nt][:, e : e + 1],
                                )
                                nc.vector.tensor_add(
                                    acc[:, sl], acc[:, sl], tmp[:, sl]
                                )
                                nc.sync.dma_start(
                                    out=out[
                                        nt * P : (nt + 1) * P,
                                        oh * FO + hh * (FO // 2)
                                        : oh * FO + (hh + 1) * (FO // 2),
                                    ],
                                    in_=acc[:, sl],
                                )
                        else:
                            emit_mul(psum, e, oh, nt)
                        if e in (1, 2) and oh == 0:
                            # bias groups (gates.T @ b_e) spread across
                            # experts 1-2, psum from the prop pool so they
                            # never contend with the expert psum rotation
                            boh, bnt = e - 1, nt
                            psum_b = prop.tile(
                                [P, FO], F32, tag="pro", name="psum_b"
                            )
                            nc.tensor.matmul(
                                psum_b,
                                gatesT_sb[bnt],
                                be_sb[:, boh * FO : (boh + 1) * FO],
                                start=True,
                                stop=True,
                            )
                            nc.vector.tensor_add(
                                acc_sb[bnt][boh], acc_sb[bnt][boh], psum_b
                            )

    legalize_single_wait(nc)
    return nc


_NC_CACHE = {}


def _get_nc():
    if "nc" not in _NC_CACHE:
        _NC_CACHE["nc"] = build_moe()
    return _NC_CACHE["nc"]


def make_in_maps(x, W_e, b_e, W_g1, b_g1, W_g2, b_g2):
    x = np.asarray(x, dtype=np.float32)
    wt = np.ascontiguousarray(
        np.asarray(W_e, dtype=np.float32).transpose(0, 2, 1)
    ).astype(BF)
    wg1t = np.ascontiguousarray(np.asarray(W_g1, dtype=np.float32).T).astype(BF)
    wg2t = np.ascontiguousarray(np.asarray(W_g2, dtype=np.float32).T).astype(BF)
    bg1 = np.asarray(b_g1, dtype=np.float32)
    bg2 = np.asarray(b_g2, dtype=np.float32).astype(BF)
    be = np.asarray(b_e, dtype=np.float32).astype(BF)
    xb = x.astype(BF)
    ident_np = np.eye(P, dtype=np.float32)
    in_maps = []
    for c in range(NCORES):
        xT_c = np.ascontiguousarray(xb[c * NLOC : (c + 1) * NLOC, :].T)
        in_maps.append(
            {
                "xT": xT_c,
                "wt": wt,
                "wg1t": wg1t,
                "wg2t": wg2t,
                "bg1": bg1,
                "bg2": bg2,
                "be": be,
                "ident": ident_np,
            }
        )
    return in_maps


def kernel(x, W_e, b_e, W_g1, b_g1, W_g2, b_g2, **run_kwargs):
    nc = _get_nc()
    in_maps = make_in_maps(x, W_e, b_e, W_g1, b_g1, W_g2, b_g2)
    res = run_bass_kernel_spmd(nc, in_maps, core_ids=list(range(NCORES)), **run_kwargs)
    out = np.concatenate([res.results[c]["out"] for c in range(NCORES)], axis=0)
    if run_kwargs:
        kernel.last_results = res
    return out


if __name__ == "__main__":
    rng = np.random.default_rng(0)
    s = 1.0 / np.sqrt(D)
    sh = 1.0 / np.sqrt(H)
    inputs = {
        "x": rng.standard_normal((N, D), dtype=np.float32),
        "W_e": rng.uniform(-s, s, (E, O, D)).astype(np.float32),
        "b_e": rng.uniform(-s, s, (E, O)).astype(np.float32),
        "W_g1": rng.uniform(-s, s, (H, D)).astype(np.float32),
        "b_g1": rng.uniform(-sh, sh, (H,)).astype(np.float32),
        "W_g2": rng.uniform(-sh, sh, (E, H)).astype(np.float32),
        "b_g2": rng.uniform(-sh, sh, (E,)).astype(np.float32),
    }
    out = kernel(**inputs)
    print("out", out.shape, out.dtype, float(np.abs(out).max()))
